# revision 11
# baseline (speedup 1.0000x reference)
"""ConvCaps dynamic-routing kernel for 8 TRN2 NeuronCores.

Strategy (data-parallel over batch B=8, one batch element per core):
  - im2col done on HOST -> x9h [72, D, 900] fp16; one DMA per pixel tile.
  - Grouped 3x3 conv as one fp16 matmul per group per pixel tile:
    stationary = im2col patches [72, npx], moving = weights [72, 512],
    PSUM out [npx, 512] -> u tile in SBUF [px, D, c, d] fp16 (Scalar copy).
  - A second, PSUM-accumulating matmul chain over the 32 groups yields
    sum_D u for free -> iteration-1 s under a zero prior (uniform c).
  - Routing runs on the Vector engine in fp16 (2x packed mode):
    multiplies with 0-stride broadcast APs; reductions as contiguous
    in-place halving add-trees (fp16 tensor_add at 2x beats 1x
    tensor_reduce). Softmax normalization is applied post-reduction
    (s = (sum_D e*u) * (1/Z)), keeping the Ln/Exp of Z off the DVE
    critical path.
  - Pixel tiles are processed in PAIRS with their routing phases
    interleaved, so one tile's DVE work hides the other's Scalar
    latencies (exp/ln handoffs).
  - All Scalar activations stay in ONE table set
    (natural_log_exp_and_others: Exp/Ln/Square/Copy/Identity);
    1/sqrt(n2+eps) is Exp(-0.5*Ln(n2+eps)) to avoid sqrt-set thrash,
    and _pin_act_tables stops the allocator from alternating sets.
  - Output s is DMA'd out as [px, (c,d)]; host transposes.
"""

import numpy as np
from contextlib import ExitStack

import concourse.bacc as bacc
import concourse.tile as tile
import concourse.mybir as mybir
from concourse.bass_utils import run_bass_kernel_spmd

F32 = mybir.dt.float32
F16 = mybir.dt.float16
AF = mybir.ActivationFunctionType

B = 8
C_IN, D_IN = 8, 32
C_OUT, D_OUT = 16, 32
KS = 3
H = W = 32
HO = WO = 30
NPX = HO * WO                 # 900 output pixels per batch element
NPXP = 1024                   # padded pixel count: 8 tiles x 128 partitions
KDIM = C_IN * KS * KS         # 72 = contraction dim of the conv matmul
CD = C_OUT * D_OUT            # 512 out-channels per group
ITERS = 3
P = 128
EPS = 1e-8
NPXT = 128                    # pixels per tile; 128 -> full partition dim
                              # and a 128-column stationary (enables FWL)


def _tree_add(nc, t, pxs, axis, n, dst):
    """Reduce t[pxs] over `axis` (1 or 2) by in-place halving adds; the
    final level writes dst."""
    while n > 2:
        h = n // 2
        if axis == 1:
            nc.vector.tensor_add(t[pxs, 0:h], t[pxs, 0:h], t[pxs, h:n])
        else:
            nc.vector.tensor_add(t[pxs, :, 0:h], t[pxs, :, 0:h], t[pxs, :, h:n])
        n = h
    if axis == 1:
        nc.vector.tensor_add(dst, t[pxs, 0], t[pxs, 1])
    else:
        nc.vector.tensor_add(dst, t[pxs, :, 0], t[pxs, :, 1])


def _conv_tile(nc, pools, st, x9h, w_sb, b0_sb, zero_prior):
    """DMA + grouped conv for one pixel tile; fills st with tiles."""
    npx, px0, pxs = st["npx"], st["px0"], st["pxs"]
    x9 = pools["x9"].tile([KDIM, D_IN, NPXT], F16, tag="x9")
    nc.sync.dma_start(x9[:, :, 0:npx], x9h[:, :, px0:px0 + npx])

    u_t = pools["u"].tile([P, D_IN, C_OUT, D_OUT], F16, tag="u")
    s_bf = pools["r2"].tile([P, C_OUT, D_OUT], F16, tag="sbf")
    if zero_prior:
        pu_s = pools["ps"].tile([P, CD], F32, tag="pus")
    for g in range(D_IN):
        pu = pools["pg"].tile([P, CD], F32, tag="pu")
        nc.tensor.matmul(pu[pxs, :], x9[:, g, pxs], w_sb[:, g, :],
                         start=True, stop=True)
        if zero_prior:
            nc.tensor.matmul(pu_s[pxs, :], x9[:, g, pxs], w_sb[:, g, :],
                             start=(g == 0), stop=(g == D_IN - 1))
        nc.scalar.copy(u_t[pxs, g], pu[pxs, :])

    r2 = pools["r2"]
    st["u"] = u_t
    st["sbf"] = s_bf
    st["b"] = r2.tile([P, D_IN, D_OUT], F32, tag="b", name="b_t")
    st["e"] = r2.tile([P, D_IN, D_OUT], F16, tag="e", name="e_t")
    st["a"] = r2.tile([P, D_IN, D_OUT], F16, tag="a", name="a_t")
    st["sq"] = r2.tile([P, C_OUT, D_OUT], F16, tag="sq", name="sq_t")
    st["su"] = r2.tile([P, C_OUT, D_OUT], F16, tag="su", name="su_t")
    st["s"] = pools["s"].tile([P, C_OUT, D_OUT], F32, tag="s", name="s_t")
    for name in ("n2", "te", "lnt", "rs", "n2p", "rcp", "f0"):
        st[name] = r2.tile([P, D_OUT], F32, tag=name, name=name + "_t")
    st["fb"] = r2.tile([P, D_OUT], F16, tag="fb", name="fb_t")
    st["z"] = r2.tile([P, D_IN], F32, tag="z", name="z_t")
    st["lz"] = r2.tile([P, D_IN], F32, tag="lz", name="lz_t")
    st["zr"] = r2.tile([P, D_IN], F16, tag="zr", name="zr_t")

    if zero_prior:
        nc.scalar.activation(s_bf[pxs], pu_s[pxs, :], AF.Copy,
                             scale=1.0 / D_IN)
    else:
        nc.scalar.copy(st["b"][pxs], b0_sb[pxs])


def _phases(nc, st, tmp, zero_prior):
    """Return the routing phase closures for one tile, in order."""
    npx, pxs = st["npx"], st["pxs"]

    def softmax():
        nc.scalar.activation(st["e"][pxs], st["b"][pxs], AF.Exp)
        # Z = sum_d e via halving tree; a_t is free scratch at this point
        nc.vector.tensor_add(st["a"][pxs, :, 0:16], st["e"][pxs, :, 0:16],
                             st["e"][pxs, :, 16:32])
        _tree_add(nc, st["a"], pxs, 2, 16, st["z"][pxs])
        nc.scalar.activation(st["lz"][pxs], st["z"][pxs], AF.Ln)
        nc.scalar.activation(st["zr"][pxs], st["lz"][pxs], AF.Exp, scale=-1.0)

    def spass(last):
        # s[c,d] = (sum_D e[D,d] * u[D,c,d]) / Z[d]
        nc.vector.tensor_mul(
            tmp[pxs], st["u"][pxs],
            st["e"][pxs].unsqueeze(2).broadcast_to((npx, D_IN, C_OUT, D_OUT)))
        _tree_add(nc, tmp, pxs, 1, D_IN, st["su"][pxs])
        dst = st["s"] if last else st["sbf"]
        nc.vector.tensor_mul(
            dst[pxs], st["su"][pxs],
            st["zr"][pxs].unsqueeze(1).broadcast_to((npx, C_OUT, D_OUT)))

    def squash_a():
        # n2 and the Scalar half of f = n2/((1+n2)*sqrt(n2+eps));
        # rsqrt via Exp(-0.5*Ln). v is folded into the a-pass.
        nc.scalar.activation(st["sq"][pxs], st["sbf"][pxs], AF.Square)
        _tree_add(nc, st["sq"], pxs, 1, C_OUT, st["n2"][pxs])
        nc.vector.tensor_scalar_add(st["te"][pxs], st["n2"][pxs], EPS)
        nc.scalar.activation(st["lnt"][pxs], st["te"][pxs], AF.Ln)
        nc.scalar.activation(st["rs"][pxs], st["lnt"][pxs], AF.Exp,
                             scale=-0.5)
        nc.scalar.add(st["n2p"][pxs], st["n2"][pxs], 1.0)

    def squash_b():
        nc.vector.reciprocal(st["rcp"][pxs], st["n2p"][pxs])
        nc.vector.tensor_mul(st["f0"][pxs], st["n2"][pxs], st["rcp"][pxs])
        nc.vector.tensor_mul(st["fb"][pxs], st["f0"][pxs], st["rs"][pxs])

    def apass(first):
        # b[D,d] (+)= f[d] * sum_c u[D,c,d] * s[c,d]
        nc.vector.tensor_mul(
            tmp[pxs], st["u"][pxs],
            st["sbf"][pxs].unsqueeze(1)
            .broadcast_to((npx, D_IN, C_OUT, D_OUT)))
        _tree_add(nc, tmp, pxs, 2, C_OUT, st["a"][pxs])
        fbb = st["fb"][pxs].unsqueeze(1).broadcast_to((npx, D_IN, D_OUT))
        if first and zero_prior:
            nc.vector.tensor_mul(st["b"][pxs], st["a"][pxs], fbb)
        else:
            nc.vector.tensor_mul(st["e"][pxs], st["a"][pxs], fbb)
            nc.vector.tensor_add(st["b"][pxs], st["b"][pxs], st["e"][pxs])

    ph = []
    for it in range(ITERS):
        first, last = it == 0, it == ITERS - 1
        if not (first and zero_prior):
            ph.append(softmax)
            ph.append(lambda last=last: spass(last))
        if last:
            break
        ph.append(squash_a)
        ph.append(squash_b)
        ph.append(lambda first=first: apass(first))
    return ph


def _body(ctx, tc, x9h, wt, b0, out, zero_prior):
    nc = tc.nc
    pools = {
        "c": ctx.enter_context(tc.tile_pool(name="consts", bufs=1)),
        "x9": ctx.enter_context(tc.tile_pool(name="x9pool", bufs=2)),
        "u": ctx.enter_context(tc.tile_pool(name="upool", bufs=2)),
        "s": ctx.enter_context(tc.tile_pool(name="spool", bufs=2)),
        "r2": ctx.enter_context(tc.tile_pool(name="rpool", bufs=2)),
        "tp": ctx.enter_context(tc.tile_pool(name="tmppool", bufs=1)),
        "pg": ctx.enter_context(tc.tile_pool(name="psum_g", bufs=4,
                                             space="PSUM")),
        "ps": ctx.enter_context(tc.tile_pool(name="psum_s", bufs=2,
                                             space="PSUM")),
    }
    w_sb = pools["c"].tile([KDIM, D_IN, CD], F16)
    nc.sync.dma_start(w_sb[:], wt)
    b0_sb = None
    if not zero_prior:
        b0_sb = pools["c"].tile([P, D_IN, D_OUT], F32)
        nc.sync.dma_start(b0_sb[:], b0)

    for pi in range(0, NPXP // NPXT, 2):
        states = []
        for t in (pi, pi + 1):
            st = {"npx": NPXT, "px0": t * NPXT, "pxs": slice(0, NPXT)}
            _conv_tile(nc, pools, st, x9h, w_sb, b0_sb, zero_prior)
            states.append(st)
        # one DVE-only scratch shared by the pair (the in-order DVE queue
        # serializes its users; sharing saves 32KB of SBUF)
        tmp = pools["tp"].tile([P, D_IN, C_OUT, D_OUT], F16, tag="tmp")
        pa, pb = [_phases(nc, st, tmp, zero_prior) for st in states]
        pa[0]()
        for i in range(1, len(pa)):
            pb[i - 1]()
            pa[i]()
        pb[-1]()
        for st in states:
            nc.sync.dma_start(out[st["px0"]:st["px0"] + st["npx"], :],
                              st["s"][st["pxs"]])


_CACHE = {}


def _pin_act_tables(arch):
    """Make natural_log_exp_and_others the only set advertising Exp/Ln so
    the act-table-load pass stops alternating between exp_and_others and
    natural_log (a ~2.7us table DMA per switch, 8x per pixel tile). The
    pinned set genuinely contains Exp/Ln/Square/Copy/Identity; set ids
    keep their act_info.json positions, so the loads stay correct."""
    import concourse.hw_specs as hw_specs
    tables = hw_specs.get_activation_tables(arch)  # functools.cache'd dict
    keep = "natural_log_exp_and_others"
    assert keep in tables
    for fn in (AF.Exp, AF.Ln):
        assert fn in tables[keep]
        for name, fns in tables.items():
            if name != keep:
                fns.discard(fn)


def _build(zero_prior: bool):
    key = ("v7", zero_prior)
    if key in _CACHE:
        return _CACHE[key]
    nc = bacc.Bacc("TRN2", target_bir_lowering=False, debug=False,
                   enable_asserts=True, num_devices=B)
    _pin_act_tables(nc.m.arch)
    x9h = nc.dram_tensor("x9h", [KDIM, D_IN, NPXP], F16,
                         kind="ExternalInput").ap()
    wt = nc.dram_tensor("wt", [KDIM, D_IN, CD], F16,
                        kind="ExternalInput").ap()
    b0 = nc.dram_tensor("b0", [P, D_IN, D_OUT], F32,
                        kind="ExternalInput").ap()
    out = nc.dram_tensor("out", [NPXP, CD], F32,
                         kind="ExternalOutput").ap()
    with tile.TileContext(nc) as tc:
        with ExitStack() as ctx:
            _body(ctx, tc, x9h, wt, b0, out, zero_prior)
    nc.compile()
    _CACHE[key] = nc
    return nc


def _prep_inputs(x, conv_w, prior):
    f16 = np.float16
    # weights: rows (D,c,d) x (C,kh,kw) -> [k=(kh,kw,C), D, (c,d)]
    wt = conv_w.reshape(D_IN, C_OUT, D_OUT, C_IN, KS, KS)
    wt = np.ascontiguousarray(wt.transpose(4, 5, 3, 0, 1, 2)).reshape(
        KDIM, D_IN, CD).astype(f16)
    pb = np.broadcast_to(prior.reshape(D_IN, D_OUT), (P, D_IN, D_OUT))
    b0 = np.ascontiguousarray(pb).astype(np.float32)
    # host-side im2col: x9[b, (kh,kw,C), D, px], zero-padded to NPXP
    x9 = np.zeros((B, KDIM, D_IN, NPXP), dtype=f16)
    for kh in range(KS):
        for kw in range(KS):
            kk = (kh * KS + kw) * C_IN
            win = x[:, :, :, kh:kh + HO, kw:kw + WO].reshape(
                B, C_IN, D_IN, NPX)
            x9[:, kk:kk + C_IN, :, 0:NPX] = win.astype(f16)
    in_maps = [
        {"x9h": np.ascontiguousarray(x9[b]), "wt": wt, "b0": b0}
        for b in range(B)
    ]
    return in_maps


def _unpack_out(arr):
    return np.ascontiguousarray(
        arr[0:NPX].reshape(HO, WO, C_OUT, D_OUT).transpose(2, 3, 0, 1))


def kernel(x, conv_w, prior):
    x = np.asarray(x, dtype=np.float32)
    conv_w = np.asarray(conv_w, dtype=np.float32)
    prior = np.asarray(prior, dtype=np.float32)
    zero_prior = not np.any(prior)
    nc = _build(zero_prior)
    in_maps = _prep_inputs(x, conv_w, prior)
    res = run_bass_kernel_spmd(nc, in_maps, list(range(B)))
    outs = [_unpack_out(res.results[b]["out"]) for b in range(B)]
    return np.stack(outs, axis=0).astype(np.float32)


# revision 12
# speedup vs baseline: 1.0093x; 1.0093x over previous
"""ConvCaps dynamic-routing kernel for 8 TRN2 NeuronCores.

Strategy (data-parallel over batch B=8, one batch element per core):
  - im2col done on HOST -> x9h [72, D, 900] fp16; one DMA per pixel tile.
  - Grouped 3x3 conv as one fp16 matmul per group per pixel tile:
    stationary = im2col patches [72, npx], moving = weights [72, 512],
    PSUM out [npx, 512] -> u tile in SBUF [px, D, c, d] fp16 (Scalar copy).
  - A second, PSUM-accumulating matmul chain over the 32 groups yields
    sum_D u for free -> iteration-1 s under a zero prior (uniform c).
  - Routing runs on the Vector engine in fp16 (2x packed mode):
    multiplies with 0-stride broadcast APs; reductions as contiguous
    in-place halving add-trees (fp16 tensor_add at 2x beats 1x
    tensor_reduce). Softmax normalization is applied post-reduction
    (s = (sum_D e*u) * (1/Z)), keeping the Ln/Exp of Z off the DVE
    critical path.
  - Pixel tiles are processed in PAIRS with their routing phases
    interleaved, so one tile's DVE work hides the other's Scalar
    latencies (exp/ln handoffs).
  - All Scalar activations stay in ONE table set
    (natural_log_exp_and_others: Exp/Ln/Square/Copy/Identity);
    1/sqrt(n2+eps) is Exp(-0.5*Ln(n2+eps)) to avoid sqrt-set thrash,
    and _pin_act_tables stops the allocator from alternating sets.
  - Output s is DMA'd out as [px, (c,d)]; host transposes.
"""

import numpy as np
from contextlib import ExitStack

import concourse.bacc as bacc
import concourse.tile as tile
import concourse.mybir as mybir
from concourse.bass_utils import run_bass_kernel_spmd

F32 = mybir.dt.float32
F16 = mybir.dt.float16
AF = mybir.ActivationFunctionType

B = 8
C_IN, D_IN = 8, 32
C_OUT, D_OUT = 16, 32
KS = 3
H = W = 32
HO = WO = 30
NPX = HO * WO                 # 900 output pixels per batch element
NPXP = 1024                   # padded pixel count: 8 tiles x 128 partitions
KDIM = C_IN * KS * KS         # 72 = contraction dim of the conv matmul
CD = C_OUT * D_OUT            # 512 out-channels per group
ITERS = 3
P = 128
EPS = 1e-8
NPXT = 128                    # pixels per tile; 128 -> full partition dim
                              # and a 128-column stationary (enables FWL)


def _tree_add(nc, t, pxs, axis, n, dst):
    """Reduce t[pxs] over `axis` (1 or 2) by in-place halving adds; the
    final level writes dst."""
    while n > 2:
        h = n // 2
        if axis == 1:
            nc.vector.tensor_add(t[pxs, 0:h], t[pxs, 0:h], t[pxs, h:n])
        else:
            nc.vector.tensor_add(t[pxs, :, 0:h], t[pxs, :, 0:h], t[pxs, :, h:n])
        n = h
    if axis == 1:
        nc.vector.tensor_add(dst, t[pxs, 0], t[pxs, 1])
    else:
        nc.vector.tensor_add(dst, t[pxs, :, 0], t[pxs, :, 1])


def _conv_tile(nc, pools, st, x9h, w_sb, b0_sb, zero_prior, use_ps):
    """DMA + grouped conv for one pixel tile; fills st with tiles.
    use_ps: accumulate sum_D u in PSUM via a second matmul chain (free
    iteration-1 s). Off for the first pair, where the extra matmuls
    would lengthen the pipeline-fill; there the DVE (idle at startup)
    computes the tree instead."""
    npx, px0, pxs = st["npx"], st["px0"], st["pxs"]
    x9 = pools["x9"].tile([KDIM, D_IN, NPXT], F16, tag="x9")
    nc.sync.dma_start(x9[:, :, 0:npx], x9h[:, :, px0:px0 + npx])

    u_t = pools["u"].tile([P, D_IN, C_OUT, D_OUT], F16, tag="u")
    s_bf = pools["r2"].tile([P, C_OUT, D_OUT], F16, tag="sbf")
    if use_ps:
        pu_s = pools["ps"].tile([P, CD], F32, tag="pus")
    for g in range(D_IN):
        pu = pools["pg"].tile([P, CD], F32, tag="pu")
        nc.tensor.matmul(pu[pxs, :], x9[:, g, pxs], w_sb[:, g, :],
                         start=True, stop=True)
        if use_ps:
            nc.tensor.matmul(pu_s[pxs, :], x9[:, g, pxs], w_sb[:, g, :],
                             start=(g == 0), stop=(g == D_IN - 1))
        nc.scalar.copy(u_t[pxs, g], pu[pxs, :])

    r2 = pools["r2"]
    st["u"] = u_t
    st["sbf"] = s_bf
    st["b"] = r2.tile([P, D_IN, D_OUT], F16, tag="b", name="b_t")
    st["e"] = r2.tile([P, D_IN, D_OUT], F16, tag="e", name="e_t")
    st["a"] = r2.tile([P, D_IN, D_OUT], F16, tag="a", name="a_t")
    st["sq"] = r2.tile([P, C_OUT, D_OUT], F16, tag="sq", name="sq_t")
    st["su"] = r2.tile([P, C_OUT, D_OUT], F16, tag="su", name="su_t")
    st["s"] = pools["s"].tile([P, C_OUT, D_OUT], F16, tag="s", name="s_t")
    for name in ("n2", "lnt", "rs", "n2p", "rcp", "f0"):
        st[name] = r2.tile([P, D_OUT], F32, tag=name, name=name + "_t")
    st["fb"] = r2.tile([P, D_OUT], F16, tag="fb", name="fb_t")
    st["z"] = r2.tile([P, D_IN], F32, tag="z", name="z_t")
    st["lz"] = r2.tile([P, D_IN], F32, tag="lz", name="lz_t")
    st["zr"] = r2.tile([P, D_IN], F16, tag="zr", name="zr_t")

    if use_ps:
        nc.scalar.activation(s_bf[pxs], pu_s[pxs, :], AF.Copy,
                             scale=1.0 / D_IN)
    if not zero_prior:
        nc.scalar.copy(st["b"][pxs], b0_sb[pxs])


def _phases(nc, st, tmp, zero_prior, use_ps):
    """Return the routing phase closures for one tile, in order."""
    npx, pxs = st["npx"], st["pxs"]

    def i0s():
        # s1 = mean_D u on the DVE (first pair only: no pu_s chain)
        nc.vector.tensor_add(tmp[pxs, 0:16], st["u"][pxs, 0:16],
                             st["u"][pxs, 16:32])
        _tree_add(nc, tmp, pxs, 1, 16, st["su"][pxs])
        nc.vector.tensor_scalar_mul(st["sbf"][pxs], st["su"][pxs],
                                    1.0 / D_IN)

    def softmax():
        nc.scalar.activation(st["e"][pxs], st["b"][pxs], AF.Exp)
        # Z = sum_d e via halving tree; a_t is free scratch at this point
        nc.vector.tensor_add(st["a"][pxs, :, 0:16], st["e"][pxs, :, 0:16],
                             st["e"][pxs, :, 16:32])
        _tree_add(nc, st["a"], pxs, 2, 16, st["z"][pxs])
        nc.scalar.activation(st["lz"][pxs], st["z"][pxs], AF.Ln)
        nc.scalar.activation(st["zr"][pxs], st["lz"][pxs], AF.Exp, scale=-1.0)

    def spass(last):
        # s[c,d] = (sum_D e[D,d] * u[D,c,d]) / Z[d]
        nc.vector.tensor_mul(
            tmp[pxs], st["u"][pxs],
            st["e"][pxs].unsqueeze(2).broadcast_to((npx, D_IN, C_OUT, D_OUT)))
        _tree_add(nc, tmp, pxs, 1, D_IN, st["su"][pxs])
        dst = st["s"] if last else st["sbf"]
        nc.vector.tensor_mul(
            dst[pxs], st["su"][pxs],
            st["zr"][pxs].unsqueeze(1).broadcast_to((npx, C_OUT, D_OUT)))

    def squash_a():
        # n2 and the Scalar half of f = n2/((1+n2)*sqrt(n2+eps));
        # rsqrt via Exp(-0.5*Ln). v is folded into the a-pass.
        nc.scalar.activation(st["sq"][pxs], st["sbf"][pxs], AF.Square)
        _tree_add(nc, st["sq"], pxs, 1, C_OUT, st["n2"][pxs])
        nc.scalar.activation(st["lnt"][pxs], st["n2"][pxs], AF.Ln,
                             bias=EPS)
        nc.scalar.activation(st["rs"][pxs], st["lnt"][pxs], AF.Exp,
                             scale=-0.5)
        nc.scalar.add(st["n2p"][pxs], st["n2"][pxs], 1.0)

    def squash_b():
        nc.vector.reciprocal(st["rcp"][pxs], st["n2p"][pxs])
        nc.vector.tensor_mul(st["f0"][pxs], st["n2"][pxs], st["rcp"][pxs])
        nc.vector.tensor_mul(st["fb"][pxs], st["f0"][pxs], st["rs"][pxs])

    def apass(first):
        # b[D,d] (+)= f[d] * sum_c u[D,c,d] * s[c,d]
        nc.vector.tensor_mul(
            tmp[pxs], st["u"][pxs],
            st["sbf"][pxs].unsqueeze(1)
            .broadcast_to((npx, D_IN, C_OUT, D_OUT)))
        _tree_add(nc, tmp, pxs, 2, C_OUT, st["a"][pxs])
        fbb = st["fb"][pxs].unsqueeze(1).broadcast_to((npx, D_IN, D_OUT))
        if first and zero_prior:
            nc.vector.tensor_mul(st["b"][pxs], st["a"][pxs], fbb)
        else:
            nc.vector.tensor_mul(st["e"][pxs], st["a"][pxs], fbb)
            nc.vector.tensor_add(st["b"][pxs], st["b"][pxs], st["e"][pxs])

    ph = []
    if zero_prior and not use_ps:
        ph.append(i0s)
    for it in range(ITERS):
        first, last = it == 0, it == ITERS - 1
        if not (first and zero_prior):
            ph.append(softmax)
            ph.append(lambda last=last: spass(last))
        if last:
            break
        ph.append(squash_a)
        ph.append(squash_b)
        ph.append(lambda first=first: apass(first))
    return ph


def _body(ctx, tc, x9h, wt, b0, out, zero_prior):
    nc = tc.nc
    pools = {
        "c": ctx.enter_context(tc.tile_pool(name="consts", bufs=1)),
        "x9": ctx.enter_context(tc.tile_pool(name="x9pool", bufs=2)),
        "u": ctx.enter_context(tc.tile_pool(name="upool", bufs=2)),
        "s": ctx.enter_context(tc.tile_pool(name="spool", bufs=2)),
        "r2": ctx.enter_context(tc.tile_pool(name="rpool", bufs=2)),
        "tp": ctx.enter_context(tc.tile_pool(name="tmppool", bufs=1)),
        "pg": ctx.enter_context(tc.tile_pool(name="psum_g", bufs=4,
                                             space="PSUM")),
        "ps": ctx.enter_context(tc.tile_pool(name="psum_s", bufs=2,
                                             space="PSUM")),
    }
    w_sb = pools["c"].tile([KDIM, D_IN, CD], F16)
    nc.sync.dma_start(w_sb[:], wt)
    b0_sb = None
    if not zero_prior:
        b0_sb = pools["c"].tile([P, D_IN, D_OUT], F32)
        nc.sync.dma_start(b0_sb[:], b0)

    for pi in range(0, NPXP // NPXT, 2):
        states = []
        use_ps = zero_prior and pi > 0
        for t in (pi, pi + 1):
            st = {"npx": NPXT, "px0": t * NPXT, "pxs": slice(0, NPXT)}
            _conv_tile(nc, pools, st, x9h, w_sb, b0_sb, zero_prior, use_ps)
            states.append(st)
        # one DVE-only scratch shared by the pair (the in-order DVE queue
        # serializes its users; sharing saves 32KB of SBUF)
        tmp = pools["tp"].tile([P, D_IN, C_OUT, D_OUT], F16, tag="tmp")
        pa, pb = [_phases(nc, st, tmp, zero_prior, use_ps)
                  for st in states]
        pa[0]()
        for i in range(1, len(pa)):
            pb[i - 1]()
            pa[i]()
        pb[-1]()
        for st in states:
            nc.sync.dma_start(out[st["px0"]:st["px0"] + st["npx"], :],
                              st["s"][st["pxs"]])


_CACHE = {}


def _pin_act_tables(arch):
    """Make natural_log_exp_and_others the only set advertising Exp/Ln so
    the act-table-load pass stops alternating between exp_and_others and
    natural_log (a ~2.7us table DMA per switch, 8x per pixel tile). The
    pinned set genuinely contains Exp/Ln/Square/Copy/Identity; set ids
    keep their act_info.json positions, so the loads stay correct."""
    import concourse.hw_specs as hw_specs
    tables = hw_specs.get_activation_tables(arch)  # functools.cache'd dict
    keep = "natural_log_exp_and_others"
    assert keep in tables
    for fn in (AF.Exp, AF.Ln):
        assert fn in tables[keep]
        for name, fns in tables.items():
            if name != keep:
                fns.discard(fn)


def _build(zero_prior: bool):
    key = ("v8", zero_prior)
    if key in _CACHE:
        return _CACHE[key]
    nc = bacc.Bacc("TRN2", target_bir_lowering=False, debug=False,
                   enable_asserts=True, num_devices=B)
    _pin_act_tables(nc.m.arch)
    eps_sb = nc.alloc_sbuf_tensor("const-f32-eps", [128, 1], F32)
    nc.gpsimd.memset(eps_sb.ap(), EPS)
    nc.const_aps.aps[(F32, EPS)] = eps_sb.ap()
    nc.all_engine_barrier()
    x9h = nc.dram_tensor("x9h", [KDIM, D_IN, NPXP], F16,
                         kind="ExternalInput").ap()
    wt = nc.dram_tensor("wt", [KDIM, D_IN, CD], F16,
                        kind="ExternalInput").ap()
    b0 = nc.dram_tensor("b0", [P, D_IN, D_OUT], F32,
                        kind="ExternalInput").ap()
    out = nc.dram_tensor("out", [NPXP, CD], F16,
                         kind="ExternalOutput").ap()
    with tile.TileContext(nc) as tc:
        with ExitStack() as ctx:
            _body(ctx, tc, x9h, wt, b0, out, zero_prior)
    nc.compile()
    _CACHE[key] = nc
    return nc


def _prep_inputs(x, conv_w, prior):
    f16 = np.float16
    # weights: rows (D,c,d) x (C,kh,kw) -> [k=(kh,kw,C), D, (c,d)]
    wt = conv_w.reshape(D_IN, C_OUT, D_OUT, C_IN, KS, KS)
    wt = np.ascontiguousarray(wt.transpose(4, 5, 3, 0, 1, 2)).reshape(
        KDIM, D_IN, CD).astype(f16)
    pb = np.broadcast_to(prior.reshape(D_IN, D_OUT), (P, D_IN, D_OUT))
    b0 = np.ascontiguousarray(pb).astype(np.float32)
    # host-side im2col: x9[b, (kh,kw,C), D, px], zero-padded to NPXP
    x9 = np.zeros((B, KDIM, D_IN, NPXP), dtype=f16)
    for kh in range(KS):
        for kw in range(KS):
            kk = (kh * KS + kw) * C_IN
            win = x[:, :, :, kh:kh + HO, kw:kw + WO].reshape(
                B, C_IN, D_IN, NPX)
            x9[:, kk:kk + C_IN, :, 0:NPX] = win.astype(f16)
    in_maps = [
        {"x9h": np.ascontiguousarray(x9[b]), "wt": wt, "b0": b0}
        for b in range(B)
    ]
    return in_maps


def _unpack_out(arr):
    return np.ascontiguousarray(
        arr[0:NPX].reshape(HO, WO, C_OUT, D_OUT).transpose(2, 3, 0, 1))


def kernel(x, conv_w, prior):
    x = np.asarray(x, dtype=np.float32)
    conv_w = np.asarray(conv_w, dtype=np.float32)
    prior = np.asarray(prior, dtype=np.float32)
    zero_prior = not np.any(prior)
    nc = _build(zero_prior)
    in_maps = _prep_inputs(x, conv_w, prior)
    res = run_bass_kernel_spmd(nc, in_maps, list(range(B)))
    outs = [_unpack_out(res.results[b]["out"]) for b in range(B)]
    return np.stack(outs, axis=0).astype(np.float32)
